# revision 14
# baseline (speedup 1.0000x reference)
"""AssociativeLIF forward scan on 8 Trainium2 NeuronCores.

Data-parallel over batch B=64 -> 8 per core. Per-core on-chip layout:
  b = b_lo*4 + b_hi  (b_lo in {0,1}, b_hi in {0..3})
  neuron d = j*64 + c  (c = cluster id = d % 64, j = d // 64)
  SBUF partition p = b_lo*64 + c   (128 partitions)
  SBUF free      f = b_hi*64 + j   (256 elements)

Cluster scatter-add  -> free-axis reduce over j (per b_hi)
Cascade mix (cf @ Wsig.T) -> one 128x128 block-diag matmul on PE
Cascade gather-back  -> 0-stride broadcast STT fused with gain scale
Refractory counter   -> implicit via notspike products:
  mask_inv(t) = n(t-1)*n(t-2), n = 1-s;  pred01 = 1-mask_inv

All f32 elementwise ops preserve the reference's rounding order exactly;
spike counts are exact small integers, so the only divergence vs the jax
CPU reference is the PE's f32 accumulation order in the 64-term cascade
matmul (~1e-7 on v, zero spike flips).
"""

import numpy as np

_T, _B, _D = 32, 64, 4096
_NC = 64
_K = _D // _NC  # 64 neurons per cluster
_NCORES = 8
_BLOC = _B // _NCORES  # 8
_VRESET = -0.1
_BIG = float(2.0 ** 20)  # q*BIG + th is exact for q=0 and dominates any v_pre


def _sigmoid_f32(x):
    x64 = np.asarray(x, dtype=np.float64)
    return np.asarray(1.0 / (1.0 + np.exp(-x64)), dtype=np.float32)


def _build(beta_s, beta_m, bm1, th_uniform):
    """th_uniform: python float for the uniform-threshold fast path,
    or None for the per-neuron threshold path (th input tensor)."""
    import concourse.bacc as bacc
    import concourse.bass as bass
    import concourse.mybir as mybir
    import concourse.tile as tile

    fp32 = mybir.dt.float32
    Alu = mybir.AluOpType
    Act = mybir.ActivationFunctionType

    nc = bacc.Bacc("TRN2", target_bir_lowering=False, debug=False,
                   num_devices=_NCORES)

    x_dram = nc.dram_tensor("x", [_T, 128, 256], fp32, kind="ExternalInput")
    if th_uniform is None:
        th_dram = nc.dram_tensor("th", [128, 256], fp32, kind="ExternalInput")
    w_dram = nc.dram_tensor("wblk", [128, 128], fp32, kind="ExternalInput")
    g_dram = nc.dram_tensor("gainT", [128, 1], fp32, kind="ExternalInput")
    bf16 = mybir.dt.bfloat16
    s_dram = nc.dram_tensor("s_out", [_T, 128, 256], bf16, kind="ExternalOutput")
    v_dram = nc.dram_tensor("v_out", [_T, 128, 256], fp32, kind="ExternalOutput")

    def bcast_j(ap2):
        """[128, 4] AP -> [128, 4, 64] AP with 0-stride j."""
        return bass.AP(tensor=ap2.tensor, offset=ap2.offset,
                       ap=[list(ap2.ap[0]), list(ap2.ap[1]), [0, _K]])

    with tile.TileContext(nc) as tc:
        with (
            tc.tile_pool(name="singles", bufs=1) as singles,
            tc.tile_pool(name="xp", bufs=6) as xp,
            tc.tile_pool(name="sp", bufs=3) as sp,
            tc.tile_pool(name="vp", bufs=3) as vp,
            tc.tile_pool(name="vprep", bufs=2) as vprep,
            tc.tile_pool(name="tmp", bufs=3) as tmpp,
            tc.tile_pool(name="cfp", bufs=2) as cfp,
            tc.tile_pool(name="psp", bufs=2, space="PSUM") as psp,
        ):
            w_t = singles.tile([128, 128], fp32)
            nc.sync.dma_start(out=w_t[:, :], in_=w_dram[:, :])
            g_t = singles.tile([128, 1], fp32)
            nc.sync.dma_start(out=g_t[:, :], in_=g_dram[:, :])
            zero_t = singles.tile([128, 256], fp32)
            nc.vector.memset(zero_t[:, :], 0.0)
            zero_bf = singles.tile([128, 256], bf16)
            nc.vector.memset(zero_bf[:, :], 0.0)
            one_t = singles.tile([128, 256], fp32)
            nc.vector.memset(one_t[:, :], 1.0)
            th0_t = singles.tile([128, 256], fp32)
            if th_uniform is None:
                nc.sync.dma_start(out=th0_t[:, :], in_=th_dram[:, :])
            else:
                nc.vector.memset(th0_t[:, :], float(th_uniform))
            i_syn = singles.tile([128, 256], fp32)
            nc.vector.memset(i_syn[:, :], 0.0)

            th_eff = th0_t       # no refractory at t=0
            pred01 = zero_t
            q_prev = zero_bf
            s_prev = zero_bf
            bv = zero_t          # beta_m * v(-1) = 0
            ps_prev = None

            for t in range(_T):
                x_t = xp.tile([128, 256], fp32, tag="x")
                nc.sync.dma_start(out=x_t[:, :], in_=x_dram[t, :, :])

                iv = i_syn[:, :].rearrange("p (b j) -> p b j", j=_K)
                if ps_prev is not None:
                    # i_syn += (mT * gain/k) broadcast over j  (fused, PSUM in)
                    nc.vector.scalar_tensor_tensor(
                        out=iv, in0=bcast_j(ps_prev[:, :]), scalar=g_t[:, :],
                        in1=iv, op0=Alu.mult, op1=Alu.add)

                # i_syn = beta_s * i_syn + x_t
                nc.vector.scalar_tensor_tensor(
                    out=i_syn[:, :], in0=i_syn[:, :], scalar=float(beta_s),
                    in1=x_t[:, :], op0=Alu.mult, op1=Alu.add)

                # v_pre = (1-beta_m)*i_syn + beta_m*v_prev (bv from ACT)
                v_pre = vprep.tile([128, 256], fp32, tag="v_pre")
                nc.vector.scalar_tensor_tensor(
                    out=v_pre[:, :], in0=i_syn[:, :], scalar=float(bm1),
                    in1=bv[:, :], op0=Alu.mult, op1=Alu.add)

                # s = (v_pre >= th_eff); th_eff = th + q*BIG (q>0 refractory)
                s = sp.tile([128, 256], bf16, tag="s")
                nc.vector.tensor_tensor(out=s[:, :], in0=v_pre[:, :],
                                        in1=th_eff[:, :], op=Alu.is_ge)
                nc.scalar.dma_start(out=s_dram[t, :, :], in_=s[:, :])

                # cf[p, b_hi] = sum_j s  (exact integer counts)
                cf = cfp.tile([128, 4], fp32, tag="cf")
                nc.vector.reduce_sum(
                    out=cf[:, :],
                    in_=s[:, :].rearrange("p (b j) -> p b j", j=_K),
                    axis=mybir.AxisListType.X)

                # mT = blockdiag(Wsig^T).T @ cf
                ps = psp.tile([128, 4], fp32, tag="ps")
                nc.tensor.matmul(ps[:, :], w_t[:, :], cf[:, :],
                                 start=True, stop=True)
                ps_prev = ps

                # ---- v-output tail (DVE, fills the matmul round-trip) ----
                # a = v_pre - s*th  (exact: s*(-th) in {0,-th})
                a = tmpp.tile([128, 256], fp32, tag="a")
                if th_uniform is not None:
                    nc.vector.scalar_tensor_tensor(
                        out=a[:, :], in0=s[:, :],
                        scalar=-float(th_uniform), in1=v_pre[:, :],
                        op0=Alu.mult, op1=Alu.add)
                else:
                    st = tmpp.tile([128, 256], fp32, tag="st")
                    nc.vector.tensor_tensor(out=st[:, :], in0=s[:, :],
                                            in1=th0_t[:, :], op=Alu.mult)
                    nc.vector.tensor_tensor(out=a[:, :], in0=v_pre[:, :],
                                            in1=st[:, :], op=Alu.subtract)
                # b = (q(t-1) == 0) * a   (fused refractory mask)
                b = tmpp.tile([128, 256], fp32, tag="b")
                nc.vector.scalar_tensor_tensor(
                    out=b[:, :], in0=q_prev[:, :], scalar=0.0,
                    in1=a[:, :], op0=Alu.is_equal, op1=Alu.mult)
                # v_new = VRESET*pred01 + b
                v_new = vp.tile([128, 256], fp32, tag="v")
                nc.vector.scalar_tensor_tensor(
                    out=v_new[:, :], in0=pred01[:, :], scalar=_VRESET,
                    in1=b[:, :], op0=Alu.mult, op1=Alu.add)
                nc.scalar.dma_start(out=v_dram[t, :, :], in_=v_new[:, :])

                # q = s + s_prev in {0,1,2}; nonzero <=> refractory next step
                q = tmpp.tile([128, 256], bf16, tag="q")
                nc.vector.tensor_tensor(out=q[:, :], in0=s[:, :],
                                        in1=s_prev[:, :], op=Alu.add)

                # ---- ACT (own SBUF ports, parallel to DVE) ----
                # bv' = beta_m * v_new
                bv = tmpp.tile([128, 256], fp32, tag="bv")
                nc.scalar.activation(out=bv[:, :], in_=v_new[:, :],
                                     func=Act.Copy, bias=0.0,
                                     scale=float(beta_m))
                # th_eff' = q*BIG + th  (exact for q=0; dominates otherwise)
                th_eff_n = tmpp.tile([128, 256], fp32, tag="th_eff")
                if th_uniform is not None:
                    nc.scalar.activation(out=th_eff_n[:, :], in_=q[:, :],
                                         func=Act.Copy,
                                         bias=float(th_uniform), scale=_BIG)
                else:
                    nc.vector.scalar_tensor_tensor(
                        out=th_eff_n[:, :], in0=q[:, :], scalar=_BIG,
                        in1=th0_t[:, :], op0=Alu.mult, op1=Alu.add)
                # pred01' = sign(q) in {0,1}
                pred01_n = tmpp.tile([128, 256], fp32, tag="pred01")
                nc.scalar.activation(out=pred01_n[:, :], in_=q[:, :],
                                     func=Act.Sign)

                q_prev = q
                s_prev = s
                pred01 = pred01_n
                th_eff = th_eff_n

    nc.compile()
    return nc


def _prep_inputs(current_in, threshold, beta_mem_raw, beta_syn_raw,
                 neighbor_weights, cluster_gain):
    """Host-side param prep + per-core layout transform."""
    f32 = np.float32
    beta_m = _sigmoid_f32(beta_mem_raw).reshape(())
    beta_s = _sigmoid_f32(beta_syn_raw).reshape(())
    bm1 = f32(1.0) - beta_m
    Wsig = _sigmoid_f32(neighbor_weights)  # (64, 64)

    wblk = np.zeros((128, 128), dtype=f32)
    wblk[0:64, 0:64] = Wsig.T
    wblk[64:128, 64:128] = Wsig.T

    gain = np.asarray(cluster_gain, dtype=f32)
    gvec = gain / f32(_K)  # exact (k = 2^6)
    gainT = np.tile(gvec, 2).reshape(128, 1).copy()

    th = np.asarray(threshold, dtype=f32)
    uniform_th = float(th.flat[0]) if np.all(th == th.flat[0]) else None
    th_jc = th.reshape(_K, _NC)  # [j, c]
    th_tile = np.ascontiguousarray(
        np.tile(th_jc.T[:, None, :], (2, 4, 1)).reshape(128, 256), dtype=f32)

    x = np.asarray(current_in, dtype=f32)
    per_core_x = []
    for core in range(_NCORES):
        xl = x[:, core * _BLOC:(core + 1) * _BLOC, :]
        xd = xl.reshape(_T, 2, 4, _K, _NC).transpose(0, 1, 4, 2, 3)
        per_core_x.append(np.ascontiguousarray(xd).reshape(_T, 128, 256))

    return (per_core_x, th_tile, wblk, gainT, uniform_th,
            float(beta_s), float(beta_m), float(bm1))


def _gather_output(dev_out):
    """(T,128,256) device layout -> (T, 8, 4096) batch-major."""
    a = dev_out.reshape(_T, 2, _NC, 4, _K).transpose(0, 1, 3, 4, 2)
    return np.ascontiguousarray(a).reshape(_T, _BLOC, _D)


def _run(current_in, threshold, beta_mem_raw, beta_syn_raw,
         neighbor_weights, cluster_gain, trace=False, tmpdir=None,
         force_general=False):
    from concourse.bass_utils import run_bass_kernel_spmd

    (per_core_x, th_tile, wblk, gainT, uniform_th, beta_s, beta_m, bm1) = \
        _prep_inputs(current_in, threshold, beta_mem_raw, beta_syn_raw,
                     neighbor_weights, cluster_gain)

    if force_general:
        uniform_th = None
    nc = _build(beta_s, beta_m, bm1, uniform_th)
    in_maps = []
    for c in range(_NCORES):
        m = {"x": per_core_x[c], "wblk": wblk, "gainT": gainT}
        if uniform_th is None:
            m["th"] = th_tile
        in_maps.append(m)

    res = run_bass_kernel_spmd(nc, in_maps, list(range(_NCORES)),
                               trace=trace, tmpdir=tmpdir)

    spikes = np.empty((_T, _B, _D), dtype=np.float32)
    v_trace = np.empty((_T, _B, _D), dtype=np.float32)
    for core in range(_NCORES):
        b0 = core * _BLOC
        spikes[:, b0:b0 + _BLOC, :] = _gather_output(
            np.asarray(res.results[core]["s_out"], dtype=np.float32))
        v_trace[:, b0:b0 + _BLOC, :] = _gather_output(res.results[core]["v_out"])
    return (spikes, v_trace), res


def kernel(current_in, threshold, beta_mem_raw, beta_syn_raw,
           neighbor_weights, cluster_gain):
    (spikes, v_trace), _ = _run(current_in, threshold, beta_mem_raw,
                                beta_syn_raw, neighbor_weights, cluster_gain)
    return spikes, v_trace


# revision 15
# speedup vs baseline: 1.1300x; 1.1300x over previous
"""AssociativeLIF forward scan on 8 Trainium2 NeuronCores.

Data-parallel over batch B=64 -> 8 per core. Per-core on-chip layout:
  b = b_lo*4 + b_hi  (b_lo in {0,1}, b_hi in {0..3})
  neuron d = j*64 + c  (c = cluster id = d % 64, j = d // 64)
  SBUF partition p = b_lo*64 + c   (128 partitions)
  SBUF free      f = b_hi*64 + j   (256 elements)

Cluster scatter-add  -> free-axis reduce over j (per b_hi)
Cascade mix (cf @ Wsig.T) -> one 128x128 block-diag matmul on PE
Cascade gather-back  -> 0-stride broadcast STT fused with gain scale
Refractory counter   -> implicit via notspike products:
  mask_inv(t) = n(t-1)*n(t-2), n = 1-s;  pred01 = 1-mask_inv

All f32 elementwise ops preserve the reference's rounding order exactly;
spike counts are exact small integers, so the only divergence vs the jax
CPU reference is the PE's f32 accumulation order in the 64-term cascade
matmul (~1e-7 on v, zero spike flips).
"""

import numpy as np

_T, _B, _D = 32, 64, 4096
_NC = 64
_K = _D // _NC  # 64 neurons per cluster
_NCORES = 8
_BLOC = _B // _NCORES  # 8
_VRESET = -0.1
_BIG = float(2.0 ** 20)  # q*BIG + th is exact for q=0 and dominates any v_pre


def _sigmoid_f32(x):
    x64 = np.asarray(x, dtype=np.float64)
    return np.asarray(1.0 / (1.0 + np.exp(-x64)), dtype=np.float32)


def _build(beta_s, beta_m, bm1, th_uniform):
    """th_uniform: python float for the uniform-threshold fast path,
    or None for the per-neuron threshold path (th input tensor)."""
    import concourse.bacc as bacc
    import concourse.bass as bass
    import concourse.mybir as mybir
    import concourse.tile as tile

    fp32 = mybir.dt.float32
    Alu = mybir.AluOpType
    Act = mybir.ActivationFunctionType

    nc = bacc.Bacc("TRN2", target_bir_lowering=False, debug=False,
                   num_devices=_NCORES)

    x_dram = nc.dram_tensor("x", [_T, 128, 256], fp32, kind="ExternalInput")
    if th_uniform is None:
        th_dram = nc.dram_tensor("th", [128, 256], fp32, kind="ExternalInput")
    w_dram = nc.dram_tensor("wblk", [128, 128], fp32, kind="ExternalInput")
    g_dram = nc.dram_tensor("gainT", [128, 1], fp32, kind="ExternalInput")
    bf16 = mybir.dt.bfloat16
    s_dram = nc.dram_tensor("s_out", [_T, 128, 256], bf16, kind="ExternalOutput")
    v_dram = nc.dram_tensor("v_out", [_T, 128, 256], fp32, kind="ExternalOutput")

    def bcast_j(ap2):
        """[128, 4] AP -> [128, 4, 64] AP with 0-stride j."""
        return bass.AP(tensor=ap2.tensor, offset=ap2.offset,
                       ap=[list(ap2.ap[0]), list(ap2.ap[1]), [0, _K]])

    with tile.TileContext(nc) as tc:
        with (
            tc.tile_pool(name="singles", bufs=1) as singles,
            tc.tile_pool(name="xp", bufs=6) as xp,
            tc.tile_pool(name="sp", bufs=3) as sp,
            tc.tile_pool(name="vp", bufs=3) as vp,
            tc.tile_pool(name="vprep", bufs=2) as vprep,
            tc.tile_pool(name="tmp", bufs=3) as tmpp,
            tc.tile_pool(name="cfp", bufs=2) as cfp,
            tc.tile_pool(name="psp", bufs=2, space="PSUM") as psp,
        ):
            w_t = singles.tile([128, 128], fp32)
            nc.sync.dma_start(out=w_t[:, :], in_=w_dram[:, :])
            g_t = singles.tile([128, 1], fp32)
            nc.sync.dma_start(out=g_t[:, :], in_=g_dram[:, :])
            zero_t = singles.tile([128, 256], fp32)
            nc.vector.memset(zero_t[:, :], 0.0)
            zero_bf = singles.tile([128, 256], bf16)
            nc.vector.memset(zero_bf[:, :], 0.0)
            one_t = singles.tile([128, 256], fp32)
            nc.vector.memset(one_t[:, :], 1.0)
            th0_t = singles.tile([128, 256], fp32)
            if th_uniform is None:
                nc.sync.dma_start(out=th0_t[:, :], in_=th_dram[:, :])
            else:
                nc.vector.memset(th0_t[:, :], float(th_uniform))
            i_syn = singles.tile([128, 256], fp32)
            nc.vector.memset(i_syn[:, :], 0.0)

            th_eff = th0_t       # no refractory at t=0
            pred01 = zero_t
            q_prev = zero_bf
            s_prev = zero_bf
            bv = zero_t          # beta_m * v(-1) = 0
            ps_prev = None

            for t in range(_T):
                x_t = xp.tile([128, 256], fp32, tag="x")
                nc.sync.dma_start(out=x_t[:, :], in_=x_dram[t, :, :])

                iv = i_syn[:, :].rearrange("p (b j) -> p b j", j=_K)
                if ps_prev is not None:
                    # i_syn += (mT * gain/k) broadcast over j  (fused, PSUM in)
                    nc.vector.scalar_tensor_tensor(
                        out=iv, in0=bcast_j(ps_prev[:, :]), scalar=g_t[:, :],
                        in1=iv, op0=Alu.mult, op1=Alu.add)

                # i_syn = beta_s * i_syn + x_t
                nc.vector.scalar_tensor_tensor(
                    out=i_syn[:, :], in0=i_syn[:, :], scalar=float(beta_s),
                    in1=x_t[:, :], op0=Alu.mult, op1=Alu.add)

                # v_pre = (1-beta_m)*i_syn + beta_m*v_prev (bv from ACT)
                v_pre = vprep.tile([128, 256], fp32, tag="v_pre")
                nc.vector.scalar_tensor_tensor(
                    out=v_pre[:, :], in0=i_syn[:, :], scalar=float(bm1),
                    in1=bv[:, :], op0=Alu.mult, op1=Alu.add)

                # s = (v_pre >= th_eff); th_eff = th + q*BIG (q>0 refractory)
                s = sp.tile([128, 256], bf16, tag="s")
                nc.vector.tensor_tensor(out=s[:, :], in0=v_pre[:, :],
                                        in1=th_eff[:, :], op=Alu.is_ge)
                nc.sync.dma_start(out=s_dram[t, :, :], in_=s[:, :])

                # cf[p, b_hi] = sum_j s  (exact integer counts)
                cf = cfp.tile([128, 4], fp32, tag="cf")
                nc.vector.reduce_sum(
                    out=cf[:, :],
                    in_=s[:, :].rearrange("p (b j) -> p b j", j=_K),
                    axis=mybir.AxisListType.X)

                # mT = blockdiag(Wsig^T).T @ cf
                ps = psp.tile([128, 4], fp32, tag="ps")
                nc.tensor.matmul(ps[:, :], w_t[:, :], cf[:, :],
                                 start=True, stop=True)
                ps_prev = ps

                # ---- v-output tail (DVE, fills the matmul round-trip) ----
                # a = v_pre - s*th  (exact: s*(-th) in {0,-th})
                a = tmpp.tile([128, 256], fp32, tag="a")
                if th_uniform is not None:
                    nc.vector.scalar_tensor_tensor(
                        out=a[:, :], in0=s[:, :],
                        scalar=-float(th_uniform), in1=v_pre[:, :],
                        op0=Alu.mult, op1=Alu.add)
                else:
                    st = tmpp.tile([128, 256], fp32, tag="st")
                    nc.vector.tensor_tensor(out=st[:, :], in0=s[:, :],
                                            in1=th0_t[:, :], op=Alu.mult)
                    nc.vector.tensor_tensor(out=a[:, :], in0=v_pre[:, :],
                                            in1=st[:, :], op=Alu.subtract)
                # b = (q(t-1) == 0) * a   (fused refractory mask)
                b = tmpp.tile([128, 256], fp32, tag="b")
                nc.vector.scalar_tensor_tensor(
                    out=b[:, :], in0=q_prev[:, :], scalar=0.0,
                    in1=a[:, :], op0=Alu.is_equal, op1=Alu.mult)
                # v_new = VRESET*pred01 + b
                v_new = vp.tile([128, 256], fp32, tag="v")
                nc.vector.scalar_tensor_tensor(
                    out=v_new[:, :], in0=pred01[:, :], scalar=_VRESET,
                    in1=b[:, :], op0=Alu.mult, op1=Alu.add)
                nc.sync.dma_start(out=v_dram[t, :, :], in_=v_new[:, :])

                # q = s + s_prev in {0,1,2}; nonzero <=> refractory next step
                q = tmpp.tile([128, 256], bf16, tag="q")
                nc.vector.tensor_tensor(out=q[:, :], in0=s[:, :],
                                        in1=s_prev[:, :], op=Alu.add)

                # ---- ACT (own SBUF ports, parallel to DVE) ----
                # bv' = beta_m * v_new
                bv = tmpp.tile([128, 256], fp32, tag="bv")
                nc.scalar.activation(out=bv[:, :], in_=v_new[:, :],
                                     func=Act.Copy, bias=0.0,
                                     scale=float(beta_m))
                # th_eff' = q*BIG + th  (exact for q=0; dominates otherwise)
                th_eff_n = tmpp.tile([128, 256], fp32, tag="th_eff")
                if th_uniform is not None:
                    nc.scalar.activation(out=th_eff_n[:, :], in_=q[:, :],
                                         func=Act.Copy,
                                         bias=float(th_uniform), scale=_BIG)
                else:
                    nc.vector.scalar_tensor_tensor(
                        out=th_eff_n[:, :], in0=q[:, :], scalar=_BIG,
                        in1=th0_t[:, :], op0=Alu.mult, op1=Alu.add)
                # pred01' = sign(q) in {0,1}
                pred01_n = tmpp.tile([128, 256], fp32, tag="pred01")
                nc.scalar.activation(out=pred01_n[:, :], in_=q[:, :],
                                     func=Act.Sign)

                q_prev = q
                s_prev = s
                pred01 = pred01_n
                th_eff = th_eff_n

    nc.compile()
    return nc


def _prep_inputs(current_in, threshold, beta_mem_raw, beta_syn_raw,
                 neighbor_weights, cluster_gain):
    """Host-side param prep + per-core layout transform."""
    f32 = np.float32
    beta_m = _sigmoid_f32(beta_mem_raw).reshape(())
    beta_s = _sigmoid_f32(beta_syn_raw).reshape(())
    bm1 = f32(1.0) - beta_m
    Wsig = _sigmoid_f32(neighbor_weights)  # (64, 64)

    wblk = np.zeros((128, 128), dtype=f32)
    wblk[0:64, 0:64] = Wsig.T
    wblk[64:128, 64:128] = Wsig.T

    gain = np.asarray(cluster_gain, dtype=f32)
    gvec = gain / f32(_K)  # exact (k = 2^6)
    gainT = np.tile(gvec, 2).reshape(128, 1).copy()

    th = np.asarray(threshold, dtype=f32)
    uniform_th = float(th.flat[0]) if np.all(th == th.flat[0]) else None
    th_jc = th.reshape(_K, _NC)  # [j, c]
    th_tile = np.ascontiguousarray(
        np.tile(th_jc.T[:, None, :], (2, 4, 1)).reshape(128, 256), dtype=f32)

    x = np.asarray(current_in, dtype=f32)
    per_core_x = []
    for core in range(_NCORES):
        xl = x[:, core * _BLOC:(core + 1) * _BLOC, :]
        xd = xl.reshape(_T, 2, 4, _K, _NC).transpose(0, 1, 4, 2, 3)
        per_core_x.append(np.ascontiguousarray(xd).reshape(_T, 128, 256))

    return (per_core_x, th_tile, wblk, gainT, uniform_th,
            float(beta_s), float(beta_m), float(bm1))


def _gather_output(dev_out):
    """(T,128,256) device layout -> (T, 8, 4096) batch-major."""
    a = dev_out.reshape(_T, 2, _NC, 4, _K).transpose(0, 1, 3, 4, 2)
    return np.ascontiguousarray(a).reshape(_T, _BLOC, _D)


def _run(current_in, threshold, beta_mem_raw, beta_syn_raw,
         neighbor_weights, cluster_gain, trace=False, tmpdir=None,
         force_general=False):
    from concourse.bass_utils import run_bass_kernel_spmd

    (per_core_x, th_tile, wblk, gainT, uniform_th, beta_s, beta_m, bm1) = \
        _prep_inputs(current_in, threshold, beta_mem_raw, beta_syn_raw,
                     neighbor_weights, cluster_gain)

    if force_general:
        uniform_th = None
    nc = _build(beta_s, beta_m, bm1, uniform_th)
    in_maps = []
    for c in range(_NCORES):
        m = {"x": per_core_x[c], "wblk": wblk, "gainT": gainT}
        if uniform_th is None:
            m["th"] = th_tile
        in_maps.append(m)

    res = run_bass_kernel_spmd(nc, in_maps, list(range(_NCORES)),
                               trace=trace, tmpdir=tmpdir)

    spikes = np.empty((_T, _B, _D), dtype=np.float32)
    v_trace = np.empty((_T, _B, _D), dtype=np.float32)
    for core in range(_NCORES):
        b0 = core * _BLOC
        spikes[:, b0:b0 + _BLOC, :] = _gather_output(
            np.asarray(res.results[core]["s_out"], dtype=np.float32))
        v_trace[:, b0:b0 + _BLOC, :] = _gather_output(res.results[core]["v_out"])
    return (spikes, v_trace), res


def kernel(current_in, threshold, beta_mem_raw, beta_syn_raw,
           neighbor_weights, cluster_gain):
    (spikes, v_trace), _ = _run(current_in, threshold, beta_mem_raw,
                                beta_syn_raw, neighbor_weights, cluster_gain)
    return spikes, v_trace
